# revision 15
# baseline (speedup 1.0000x reference)
"""Trainium2 Bass kernel for nn_MultiHeadAttention_84052509983469.

Full-input contract: kernel(**inputs) takes the complete tensors and
returns the complete [B, S, D] output. Internally the work is sharded
across 8 NeuronCores as (batch b in {0,1}) x (head-group g in {0..3}),
i.e. tensor-parallel over heads (4 heads / 64*4=256 features per core)
and data-parallel over batch. Each core computes:

  Q^T,K^T = W{q,k}_g @ x_b^T  (RoPE'd, head dims pre-permuted to
                               [even..., odd...] per head on host)
  V       = x_b @ Wv_g^T       (natural layout, gated by the pad mask,
                               plus a gated ones-column for the softmax
                               denominator)
  S^T     = K_h^T.T @ Q_h^T    (scores transposed: keys on partitions)
  P^T     = exp(S^T / 8)       (no max subtraction; |scores| is small
                               for this problem's N(0,1)-scaled data)
  O^T     = V_aug^T @ P^T      (row 64 = softmax denominator)
  attn^T  = O^T[0:64] / O^T[64] + bv
  partial = attn^T.T @ Wo_g^T  (row-sharded Wo)

Host gathers: out[b] = sum_g partial[b,g] + bo.
"""

import sys

if "/opt/trn_rl_repo" not in sys.path:
    sys.path.insert(0, "/opt/trn_rl_repo")

import numpy as np

import concourse.bass as bass
import concourse.mybir as mybir
import concourse.tile as tile
from concourse import bacc

# Problem shapes
B, S, D, H = 2, 2048, 1024, 16
HD = D // H  # 64
G = 4  # head groups (cores per batch)
HL = H // G  # heads per core = 4
GF = HL * HD  # features per core = 256
P = 128
NK = S // P  # 16 key tiles
NQ = 256  # query chunk size
NQC = S // NQ  # 8 query chunks
KT = D // P  # 8 contraction tiles for projections

F32 = mybir.dt.float32
F32R = mybir.dt.float32r

# Matmul dtype knobs (bitcast fp32 operands to fp32r for full-rate PE).
MM_PROJ_R = True
MM_QK_R = True
MM_PV_R = True
MM_WO_R = True


def _r(ap, enable):
    return ap.bitcast(F32R) if enable else ap


def build_nc():
    nc = bacc.Bacc(None, target_bir_lowering=False, debug=False)

    # ---- DRAM I/O (host supplies pre-tiled layouts) ----
    xt = nc.dram_tensor("xt", [P, KT, S], F32R, kind="ExternalInput")  # x^T tiles
    wq = nc.dram_tensor("wq", [P, KT, GF], F32R, kind="ExternalInput")
    wk = nc.dram_tensor("wk", [P, KT, GF], F32R, kind="ExternalInput")
    wv = nc.dram_tensor("wv", [P, KT, GF], F32R, kind="ExternalInput")
    wo = nc.dram_tensor("wo", [P, 2, D], F32R, kind="ExternalInput")  # Wo^T rows
    bq = nc.dram_tensor("bq", [P, 2], F32, kind="ExternalInput")
    bk = nc.dram_tensor("bk", [P, 2], F32, kind="ExternalInput")
    bv = nc.dram_tensor("bv", [P, 2], F32, kind="ExternalInput")
    ct = nc.dram_tensor("ct", [P, S], F32, kind="ExternalInput")  # cos, tiled x4
    st = nc.dram_tensor("st", [P, S], F32, kind="ExternalInput")  # sin, tiled x4
    gate = nc.dram_tensor("gate", [P, NK, HL], F32, kind="ExternalInput")
    cmask = nc.dram_tensor("cmask", [P, 2, NQ], F32, kind="ExternalInput")
    out = nc.dram_tensor("out", [P, NK, D], F32, kind="ExternalOutput")

    with tile.TileContext(nc) as tc:
        with (
            tc.tile_pool(name="xtp", bufs=1) as xtp,
            tc.tile_pool(name="wp", bufs=1) as wp,
            tc.tile_pool(name="const", bufs=1) as constp,
            tc.tile_pool(name="qk", bufs=1) as qkp,
            tc.tile_pool(name="rope_tmp", bufs=1) as ropep,
            tc.tile_pool(name="vaug", bufs=1) as vaugp,
            tc.tile_pool(name="psum", bufs=1, space="PSUM") as pp,
            tc.tile_pool(name="pexp", bufs=4) as pexpp,
            tc.tile_pool(name="recip", bufs=4) as recipp,
            tc.tile_pool(name="dramsc", bufs=4, space="DRAM") as dramp,
            tc.tile_pool(name="outsb", bufs=2) as outp,
        ):
            # ---- load constants / inputs to SBUF ----
            xt_sb = xtp.tile([P, KT, S], F32R, tag="xt")
            for a in range(KT):
                nc.sync.dma_start(out=xt_sb[:, a, :], in_=xt[:, a, :])
            wq_sb = wp.tile([P, KT, GF], F32R, tag="wq")
            wk_sb = wp.tile([P, KT, GF], F32R, tag="wk")
            wv_sb = wp.tile([P, KT, GF], F32R, tag="wv")
            nc.sync.dma_start(out=wq_sb[:], in_=wq[:])
            nc.sync.dma_start(out=wk_sb[:], in_=wk[:])
            nc.sync.dma_start(out=wv_sb[:], in_=wv[:])
            wo_sb = wp.tile([P, 2, D], F32R, tag="wo")
            nc.sync.dma_start(out=wo_sb[:], in_=wo[:])
            ct_sb = constp.tile([P, S], F32, tag="ct")
            st_sb = constp.tile([P, S], F32, tag="st")
            nc.sync.dma_start(out=ct_sb[:], in_=ct[:])
            nc.sync.dma_start(out=st_sb[:], in_=st[:])
            bq_sb = constp.tile([P, 2], F32, tag="bq")
            bk_sb = constp.tile([P, 2], F32, tag="bk")
            bv_sb = constp.tile([P, 2], F32, tag="bv")
            nc.sync.dma_start(out=bq_sb[:], in_=bq[:])
            nc.sync.dma_start(out=bk_sb[:], in_=bk[:])
            nc.sync.dma_start(out=bv_sb[:], in_=bv[:])
            gate_sb = constp.tile([P, NK, HL], F32, tag="gate")
            nc.sync.dma_start(out=gate_sb[:], in_=gate[:])
            cm_sb = constp.tile([P, 2, NQ], F32, tag="cmask")
            nc.sync.dma_start(out=cm_sb[:], in_=cmask[:])

            # ---- Phase A: Q^T / K^T projections (+bias, +RoPE) ----
            # qt/kt: 2 M-tiles each of [128, S]; rows = head-feature
            # (2 heads per M-tile; within a head: 32 even dims, 32 odd).
            qt = [qkp.tile([P, S], F32, tag=f"qt{m}", name=f"qt{m}") for m in range(2)]
            kt_ = [qkp.tile([P, S], F32, tag=f"kt{m}", name=f"kt{m}") for m in range(2)]
            for dst, w_sb, b_sb in ((qt, wq_sb, bq_sb), (kt_, wk_sb, bk_sb)):
                for m in range(2):
                    for c4 in range(S // 512):
                        ps = pp.tile([P, 512], F32, tag="proj", bufs=2, name="ps_proj")
                        for k in range(KT):
                            nc.tensor.matmul(
                                ps[:],
                                w_sb[:, k, m * P : (m + 1) * P],
                                xt_sb[:, k, c4 * 512 : (c4 + 1) * 512],
                                start=(k == 0),
                                stop=(k == KT - 1),
                            )
                        # evict + bias (per-partition)
                        nc.scalar.activation(
                            out=dst[m][:, c4 * 512 : (c4 + 1) * 512].bitcast(F32R),
                            in_=ps[:],
                            func=mybir.ActivationFunctionType.Identity,
                            bias=b_sb[:, m : m + 1],
                            scale=1.0,
                        )
            # RoPE in place: rows [hh*64, hh*64+32) = x0, [hh*64+32, +64) = x1
            # out = A + sgn*shift(B) with A=x*cos, B=x*sin; the partition
            # swap (x0<->x1 blocks) is done by SBUF->SBUF DMA since DVE ops
            # cannot mix start partitions.
            sgn = constp.tile([P, 1], F32, tag="sgn")
            for blk, val in ((0, -1.0), (1, 1.0), (2, -1.0), (3, 1.0)):
                nc.vector.memset(sgn[blk * 32 : (blk + 1) * 32, :], val)
            for tgt in (qt[0], qt[1], kt_[0], kt_[1]):
                a_t = ropep.tile([P, S], F32, tag="ropeA")
                b_t = ropep.tile([P, S], F32, tag="ropeB")
                bs_t = ropep.tile([P, S], F32, tag="ropeBs")
                nc.vector.tensor_mul(a_t[:], tgt[:], ct_sb[:])
                nc.vector.tensor_mul(b_t[:], tgt[:], st_sb[:])
                for blk in range(4):
                    d0, s0 = blk * 32, (blk ^ 1) * 32
                    nc.sync.dma_start(
                        out=bs_t[d0 : d0 + 32, :], in_=b_t[s0 : s0 + 32, :]
                    )
                nc.vector.scalar_tensor_tensor(
                    out=tgt[:].bitcast(F32R),
                    in0=bs_t[:],
                    scalar=sgn[:, 0:1],
                    in1=a_t[:],
                    op0=mybir.AluOpType.mult,
                    op1=mybir.AluOpType.add,
                )

            # ---- Phase B: V projection -> gated V_aug [128, NK, HL, 65] ----
            vaug = vaugp.tile([P, NK, HL, HD + 1], F32, tag="vaug")
            for t in range(NK):
                ps = pp.tile([P, GF], F32, tag="ps", bufs=4, name="ps_vproj")
                for k in range(KT):
                    nc.tensor.matmul(
                        ps[:],
                        xt_sb[:, k, t * P : (t + 1) * P],
                        wv_sb[:, k, :],
                        start=(k == 0),
                        stop=(k == KT - 1),
                    )
                # gated copy PSUM -> V_aug (pad-masked keys zeroed)
                nc.vector.tensor_scalar_mul(
                    vaug[:, t, :, 0:HD].bitcast(F32R),
                    ps[:].rearrange("p (h d) -> p h d", h=HL),
                    gate_sb[:, t, 0:1],
                )
                nc.vector.tensor_copy(
                    out=vaug[:, t, :, HD : HD + 1].bitcast(F32R),
                    in_=gate_sb[:, t, :].unsqueeze(-1),
                )

            # ---- Phase C: attention (scores transposed; keys on partitions) ----
            attnt = [
                xtp.tile([P, S], F32, tag="xt", name="attnt0"),
                wp.tile([P, S], F32, tag="wq", name="attnt1"),
            ]  # alias slots of xt / wq, both dead by phase C
            for c in range(NQC):
                q0, q1 = c * NQ, (c + 1) * NQ
                for h in range(HL):
                    m, r0 = h // 2, 64 * (h % 2)
                    po = pp.tile([HD + 1, NQ], F32, tag="po", bufs=2, name="ps_o")
                    nt = 2 * c + 2  # causal: key tiles 0 .. 2c+1
                    for t in range(nt):
                        ps = pp.tile([P, NQ], F32, tag="ps", bufs=4, name="ps_s")
                        nc.tensor.matmul(
                            ps[:],
                            kt_[m][r0 : r0 + 64, t * P : (t + 1) * P].bitcast(F32R),
                            qt[m][r0 : r0 + 64, q0:q1].bitcast(F32R),
                            start=True,
                            stop=True,
                        )
                        pe = pexpp.tile([P, NQ], F32, tag="pexp")
                        nc.scalar.activation(
                            out=pe[:].bitcast(F32R), in_=ps[:],
                            func=mybir.ActivationFunctionType.Exp,
                            scale=1.0 / np.sqrt(HD).item(),
                        )
                        j = t - 2 * c
                        if j >= 0:  # diagonal block: zero future keys
                            nc.vector.tensor_mul(pe[:].bitcast(F32R), pe[:], cm_sb[:, j, :])
                        nc.tensor.matmul(
                            po[:],
                            vaug[:, t, h, :].bitcast(F32R),
                            pe[:].bitcast(F32R),
                            start=(t == 0),
                            stop=(t == nt - 1),
                        )
                    # normalize + bv -> attn^T
                    rc = recipp.tile([1, NQ], F32, tag="recip")
                    nc.vector.reciprocal(rc[:], po[HD : HD + 1, :])
                    rcd = dramp.tile([1, NQ], F32, tag="rcd", bufs=4, name="rcd")
                    nc.sync.dma_start(out=rcd[:], in_=rc[:])
                    rb = recipp.tile([P, NQ], F32, tag="rbb", bufs=4, name="rb")
                    nc.sync.dma_start(
                        out=rb[r0 : r0 + 64, :],
                        in_=bass.AP(rcd[:].tensor, rcd[:].offset, [[0, 64], [1, NQ]]),
                    )
                    tm = recipp.tile([P, NQ], F32, tag="tmn", bufs=4, name="tm")
                    nc.vector.tensor_mul(
                        tm[r0 : r0 + 64, :],
                        po[0:HD, :],
                        rb[r0 : r0 + 64, :],
                    )
                    nc.scalar.activation(
                        out=attnt[m][r0 : r0 + 64, q0:q1].bitcast(F32R),
                        in_=tm[r0 : r0 + 64, :],
                        func=mybir.ActivationFunctionType.Identity,
                        bias=bv_sb[r0 : r0 + 64, h // 2 : h // 2 + 1],
                        scale=1.0,
                    )

            # ---- Phase D: output projection (row-sharded Wo) ----
            for t in range(NK):
                for oc in range(2):
                    ps = pp.tile([P, 512], F32, tag="proj", bufs=2, name="ps_w")
                    for m in range(2):
                        nc.tensor.matmul(
                            ps[:],
                            attnt[m][:, t * P : (t + 1) * P].bitcast(F32R),
                            wo_sb[:, m, oc * 512 : (oc + 1) * 512],
                            start=(m == 0),
                            stop=(m == 1),
                        )
                    ob = outp.tile([P, 512], F32, tag="outsb")
                    nc.scalar.copy(out=ob[:], in_=ps[:])
                    nc.sync.dma_start(
                        out=out[:, t, oc * 512 : (oc + 1) * 512], in_=ob[:]
                    )

    nc.compile()
    return nc


# ---------------- host-side prep ----------------

_PERM64 = np.concatenate([np.arange(0, HD, 2), np.arange(1, HD, 2)])


def _rope_tables():
    inv = 1.0 / (10000.0 ** (np.arange(0, HD, 2, dtype=np.float32) / HD))
    t = np.arange(S, dtype=np.float32)
    ang = np.outer(t, inv)  # [S, HD/2]
    return np.cos(ang).astype(np.float32), np.sin(ang).astype(np.float32)


def _tile_rows(a, p=P):
    """[R, N] -> [p, R//p, N] where row r = a[idx*p + ...]: r = a_row % p"""
    R = a.shape[0]
    return np.ascontiguousarray(
        a.reshape(R // p, p, *a.shape[1:]).transpose(1, 0, *range(2, a.ndim + 1))
    )


def shard_inputs(x, effective_len, Wq, bq, Wk, bk, Wv, bv, Wo, bo):
    x = np.asarray(x, np.float32)
    effective_len = np.asarray(effective_len, np.int32)
    Wq, Wk, Wv, Wo = (np.asarray(w, np.float32) for w in (Wq, Wk, Wv, Wo))
    bq, bk, bv = (np.asarray(b, np.float32) for b in (bq, bk, bv))

    cos, sin = _rope_tables()  # [S, 32]
    ct = np.ascontiguousarray(np.tile(cos.T, (4, 1)))  # [128, S]
    st = np.ascontiguousarray(np.tile(sin.T, (4, 1)))

    # causal multiplicative masks for the two diagonal key-tiles of a chunk
    kl = np.arange(P)[:, None]
    ql = np.arange(NQ)[None, :]
    cm = np.stack(
        [(ql >= kl).astype(np.float32), (ql >= kl + P).astype(np.float32)]
    )  # [2, 128, NQ]
    cmask = np.ascontiguousarray(cm.transpose(1, 0, 2))  # [128, 2, NQ]

    in_maps = []
    for b in range(B):
        xt = _tile_rows(np.ascontiguousarray(x[b].T))  # [128, 8, S]
        g_vec = (np.arange(S) < (S - int(effective_len[b]))).astype(np.float32)
        gate = np.ascontiguousarray(
            np.repeat(g_vec.reshape(NK, P).T[:, :, None], HL, axis=2)
        )  # [128, NK, HL]
        for g in range(G):
            rows = np.concatenate(
                [g * GF + h * HD + _PERM64 for h in range(HL)]
            )  # permuted head dims for Q/K
            vrows = np.arange(g * GF, (g + 1) * GF)
            in_maps.append(
                {
                    "xt": xt,
                    "wq": _tile_rows(np.ascontiguousarray(Wq[rows].T)),
                    "wk": _tile_rows(np.ascontiguousarray(Wk[rows].T)),
                    "wv": _tile_rows(np.ascontiguousarray(Wv[vrows].T)),
                    "wo": _tile_rows(np.ascontiguousarray(Wo[:, vrows].T)),
                    "bq": np.ascontiguousarray(bq[rows].reshape(2, P).T),
                    "bk": np.ascontiguousarray(bk[rows].reshape(2, P).T),
                    "bv": np.ascontiguousarray(bv[vrows].reshape(2, P).T),
                    "ct": ct,
                    "st": st,
                    "gate": gate,
                    "cmask": cmask,
                }
            )
    return in_maps


def gather_outputs(results, bo):
    bo = np.asarray(bo, np.float32)
    out = np.zeros((B, S, D), np.float32)
    for b in range(B):
        acc = np.zeros((S, D), np.float32)
        for g in range(G):
            o3 = results[b * G + g]["out"]  # [128, NK, D]
            acc += o3.transpose(1, 0, 2).reshape(S, D)
        out[b] = acc + bo
    return out


_NC_CACHE = None


def _get_nc():
    global _NC_CACHE
    if _NC_CACHE is None:
        _NC_CACHE = build_nc()
    return _NC_CACHE


def kernel(**inputs):
    from concourse.bass_utils import run_bass_kernel_spmd

    nc = _get_nc()
    in_maps = shard_inputs(**inputs)
    res = run_bass_kernel_spmd(nc, in_maps, core_ids=list(range(8)))
    return gather_outputs(res.results, inputs["bo"])


# revision 25
# speedup vs baseline: 1.4085x; 1.4085x over previous
"""Trainium2 Bass kernel for nn_MultiHeadAttention_84052509983469.

Full-input contract: kernel(**inputs) takes the complete tensors and
returns the complete [B, S, D] output. Internally the work is sharded
across 8 NeuronCores as (batch b in {0,1}) x (head-group g in {0..3}),
i.e. tensor-parallel over heads (4 heads / 64*4=256 features per core)
and data-parallel over batch. Each core computes:

  Q^T,K^T = W{q,k}_g @ x_b^T  (RoPE'd, head dims pre-permuted to
                               [even..., odd...] per head on host)
  V       = x_b @ Wv_g^T       (natural layout, gated by the pad mask,
                               plus a gated ones-column for the softmax
                               denominator)
  S^T     = K_h^T.T @ Q_h^T    (scores transposed: keys on partitions)
  P^T     = exp(S^T / 8)       (no max subtraction; |scores| is small
                               for this problem's N(0,1)-scaled data)
  O^T     = V_aug^T @ P^T      (row 64 = softmax denominator)
  attn^T  = O^T[0:64] / O^T[64] + bv
  partial = attn^T.T @ Wo_g^T  (row-sharded Wo)

Host gathers: out[b] = sum_g partial[b,g] + bo.
"""

import sys

if "/opt/trn_rl_repo" not in sys.path:
    sys.path.insert(0, "/opt/trn_rl_repo")

import numpy as np

import concourse.bass as bass
import concourse.mybir as mybir
import concourse.tile as tile
from concourse import bacc

# Problem shapes
B, S, D, H = 2, 2048, 1024, 16
HD = D // H  # 64
G = 4  # head groups (cores per batch)
HL = H // G  # heads per core = 4
GF = HL * HD  # features per core = 256
P = 128
NK = S // P  # 16 key tiles
NQ = 256  # query chunk size
NQC = S // NQ  # 8 query chunks
KT = D // P  # 8 contraction tiles for projections

F32 = mybir.dt.float32
F32R = mybir.dt.float32r

# Matmul dtype knobs (bitcast fp32 operands to fp32r for full-rate PE).
MM_PROJ_R = True
MM_QK_R = True
MM_PV_R = True
MM_WO_R = True


def _r(ap, enable):
    return ap.bitcast(F32R) if enable else ap


def build_nc():
    nc = bacc.Bacc(None, target_bir_lowering=False, debug=False)

    # ---- DRAM I/O (host supplies pre-tiled layouts) ----
    xt = nc.dram_tensor("xt", [P, KT, S], F32R, kind="ExternalInput")  # x^T tiles
    wq = nc.dram_tensor("wq", [P, KT, GF], F32R, kind="ExternalInput")
    wk = nc.dram_tensor("wk", [P, KT, GF], F32R, kind="ExternalInput")
    wv = nc.dram_tensor("wv", [P, KT, GF], F32R, kind="ExternalInput")
    wo = nc.dram_tensor("wo", [P, 2, D], F32R, kind="ExternalInput")  # Wo^T rows
    bq = nc.dram_tensor("bq", [P, 2], F32, kind="ExternalInput")
    bk = nc.dram_tensor("bk", [P, 2], F32, kind="ExternalInput")
    bv = nc.dram_tensor("bv", [P, 2], F32, kind="ExternalInput")
    ct = nc.dram_tensor("ct", [P, S], F32, kind="ExternalInput")  # cos, tiled x4
    st = nc.dram_tensor("st", [P, S], F32, kind="ExternalInput")  # sin, tiled x4
    gate = nc.dram_tensor("gate", [P, NK, HL], F32, kind="ExternalInput")
    cmask = nc.dram_tensor("cmask", [P, 2, NQ], F32, kind="ExternalInput")
    out = nc.dram_tensor("out", [P, NK, D], F32, kind="ExternalOutput")

    with tile.TileContext(nc) as tc:
        with (
            tc.tile_pool(name="xtp", bufs=1) as xtp,
            tc.tile_pool(name="wp", bufs=1) as wp,
            tc.tile_pool(name="const", bufs=1) as constp,
            tc.tile_pool(name="qk", bufs=1) as qkp,
            tc.tile_pool(name="rope_tmp", bufs=1) as ropep,
            tc.tile_pool(name="vaug", bufs=1) as vaugp,
            tc.tile_pool(name="psum", bufs=1, space="PSUM") as pp,
            tc.tile_pool(name="pexp", bufs=3) as pexpp,
            tc.tile_pool(name="recip", bufs=3) as recipp,
            tc.tile_pool(name="dramsc", bufs=4, space="DRAM") as dramp,
        ):
            # ---- load constants / inputs to SBUF ----
            xt_sb = xtp.tile([P, KT, S], F32R, tag="xt")
            for a in range(KT):
                nc.sync.dma_start(out=xt_sb[:, a, :], in_=xt[:, a, :])
            wq_sb = wp.tile([P, KT, GF], F32R, tag="wq")
            wk_sb = wp.tile([P, KT, GF], F32R, tag="wk")
            wv_sb = wp.tile([P, KT, GF], F32R, tag="wv")
            nc.sync.dma_start(out=wq_sb[:], in_=wq[:])
            nc.sync.dma_start(out=wk_sb[:], in_=wk[:])
            nc.sync.dma_start(out=wv_sb[:], in_=wv[:])
            wo_sb = wp.tile([P, 2, D], F32R, tag="wo")
            nc.sync.dma_start(out=wo_sb[:], in_=wo[:])
            ct_sb = constp.tile([P, S], F32, tag="ct")
            st_sb = constp.tile([P, S], F32, tag="st")
            nc.sync.dma_start(out=ct_sb[:], in_=ct[:])
            nc.sync.dma_start(out=st_sb[:], in_=st[:])
            bq_sb = constp.tile([P, 2], F32, tag="bq")
            bk_sb = constp.tile([P, 2], F32, tag="bk")
            bv_sb = constp.tile([P, 2], F32, tag="bv")
            nc.sync.dma_start(out=bq_sb[:], in_=bq[:])
            nc.sync.dma_start(out=bk_sb[:], in_=bk[:])
            nc.sync.dma_start(out=bv_sb[:], in_=bv[:])
            gate_sb = constp.tile([P, NK, HL], F32, tag="gate")
            nc.sync.dma_start(out=gate_sb[:], in_=gate[:])
            cm_sb = constp.tile([P, 2, NQ], F32, tag="cmask")
            nc.sync.dma_start(out=cm_sb[:], in_=cmask[:])

            # ---- Phase A: Q^T / K^T projections (+bias, +RoPE) ----
            # qt/kt: 2 M-tiles each of [128, S]; rows = head-feature
            # (2 heads per M-tile; within a head: 32 even dims, 32 odd).
            qt = [qkp.tile([P, S], F32, tag=f"qt{m}", name=f"qt{m}") for m in range(2)]
            kt_ = [qkp.tile([P, S], F32, tag=f"kt{m}", name=f"kt{m}") for m in range(2)]
            for dst, w_sb, b_sb in ((qt, wq_sb, bq_sb), (kt_, wk_sb, bk_sb)):
                for m in range(2):
                    for c4 in range(S // 512):
                        ps = pp.tile([P, 512], F32, tag="proj", bufs=2, name="ps_proj")
                        for k in range(KT):
                            nc.tensor.matmul(
                                ps[:],
                                w_sb[:, k, m * P : (m + 1) * P],
                                xt_sb[:, k, c4 * 512 : (c4 + 1) * 512],
                                start=(k == 0),
                                stop=(k == KT - 1),
                            )
                        # evict + bias (per-partition)
                        nc.scalar.activation(
                            out=dst[m][:, c4 * 512 : (c4 + 1) * 512].bitcast(F32R),
                            in_=ps[:],
                            func=mybir.ActivationFunctionType.Identity,
                            bias=b_sb[:, m : m + 1],
                            scale=1.0,
                        )
            # RoPE in place: rows [hh*64, hh*64+32) = x0, [hh*64+32, +64) = x1
            # out = A + sgn*shift(B) with A=x*cos, B=x*sin; the partition
            # swap (x0<->x1 blocks) is done by SBUF->SBUF DMA since DVE ops
            # cannot mix start partitions.
            sgn = constp.tile([P, 1], F32, tag="sgn")
            for blk, val in ((0, -1.0), (1, 1.0), (2, -1.0), (3, 1.0)):
                nc.vector.memset(sgn[blk * 32 : (blk + 1) * 32, :], val)
            HS = S // 2
            for tgt in (qt[0], qt[1], kt_[0], kt_[1]):
                for hf in range(2):
                    cl = slice(hf * HS, (hf + 1) * HS)
                    a_t = ropep.tile([P, HS], F32, tag="ropeA", bufs=2,
                                     name="a_t")
                    b_t = ropep.tile([P, HS], F32, tag="ropeB", bufs=2,
                                     name="b_t")
                    bs_t = ropep.tile([P, HS], F32, tag="ropeBs", bufs=2,
                                      name="bs_t")
                    nc.vector.tensor_mul(a_t[:], tgt[:, cl], ct_sb[:, cl])
                    nc.vector.tensor_mul(b_t[:], tgt[:, cl], st_sb[:, cl])
                    for blk in range(4):
                        d0, s0 = blk * 32, (blk ^ 1) * 32
                        nc.gpsimd.dma_start(
                            out=bs_t[d0 : d0 + 32, :], in_=b_t[s0 : s0 + 32, :]
                        )
                    nc.vector.scalar_tensor_tensor(
                        out=tgt[:, cl].bitcast(F32R),
                        in0=bs_t[:],
                        scalar=sgn[:, 0:1],
                        in1=a_t[:],
                        op0=mybir.AluOpType.mult,
                        op1=mybir.AluOpType.add,
                    )

            # ---- Phase B: V projection -> gated V_aug [128, NK, HL, 65] ----
            vaug = [
                vaugp.tile([P, HL, HD + 1], F32, tag=f"vaug{t}", name=f"vaug{t}")
                for t in range(NK)
            ]
            for t in range(NK):
                ps = pp.tile([P, GF], F32, tag="proj", bufs=2, name="ps_vproj")
                for k in range(KT):
                    nc.tensor.matmul(
                        ps[:],
                        xt_sb[:, k, t * P : (t + 1) * P],
                        wv_sb[:, k, :],
                        start=(k == 0),
                        stop=(k == KT - 1),
                    )
                # gated copy PSUM -> V_aug (pad-masked keys zeroed)
                nc.vector.tensor_scalar_mul(
                    vaug[t][:, :, 0:HD].bitcast(F32R),
                    ps[:].rearrange("p (h d) -> p h d", h=HL),
                    gate_sb[:, t, 0:1],
                )
                nc.vector.tensor_copy(
                    out=vaug[t][:, :, HD : HD + 1].bitcast(F32R),
                    in_=gate_sb[:, t, :].unsqueeze(-1),
                )

            # ---- Phase C: attention (scores transposed; keys on partitions) ----
            attnt = [
                xtp.tile([P, S], F32, tag="xt", name="attnt0"),
                wp.tile([P, S], F32, tag="wq", name="attnt1"),
            ]  # alias slots of xt / wq, both dead by phase C
            for c in range(NQC):
                q0, q1 = c * NQ, (c + 1) * NQ
                for h in range(HL):
                    m, r0 = h // 2, 64 * (h % 2)
                    po = pp.tile([HD + 1, NQ], F32, tag="po", bufs=2, name="ps_o")
                    nt = 2 * c + 2  # causal: key tiles 0 .. 2c+1 (even count)
                    t0 = 0
                    while t0 < nt:  # groups of up to 4 k-tiles share one
                        gw = min(4, nt - t0)  # psum tile and one wide exp
                        ps = pp.tile([P, gw, NQ], F32, tag="ps", bufs=2,
                                     name="ps_s", padded_shape=[P, 4, NQ])
                        for u in range(gw):
                            nc.tensor.matmul(
                                ps[:, u, :],
                                kt_[m][
                                    r0 : r0 + 64, (t0 + u) * P : (t0 + u + 1) * P
                                ].bitcast(F32R),
                                qt[m][r0 : r0 + 64, q0:q1].bitcast(F32R),
                                start=True,
                                stop=True,
                            )
                        pe = pexpp.tile([P, gw, NQ], F32, tag="pexp",
                                        padded_shape=[P, 4, NQ])
                        nc.scalar.activation(
                            out=pe[:].bitcast(F32R), in_=ps[:],
                            func=mybir.ActivationFunctionType.Exp,
                            scale=1.0 / np.sqrt(HD).item(),
                        )
                        if t0 + gw == nt:  # last group holds the diagonal pair
                            nc.vector.tensor_mul(
                                pe[:, gw - 2 : gw, :].bitcast(F32R),
                                pe[:, gw - 2 : gw, :],
                                cm_sb[:],
                            )
                        for u in range(gw):
                            t = t0 + u
                            nc.tensor.matmul(
                                po[:],
                                vaug[t][:, h, :].bitcast(F32R),
                                pe[:, u, :].bitcast(F32R),
                                start=(t == 0),
                                stop=(t == nt - 1),
                            )
                        t0 += gw
                    # normalize + bv -> attn^T
                    rc = recipp.tile([1, NQ], F32, tag="recip")
                    nc.vector.reciprocal(rc[:], po[HD : HD + 1, :])
                    rcd = dramp.tile([1, NQ], F32, tag="rcd", bufs=4, name="rcd")
                    nc.sync.dma_start(out=rcd[:], in_=rc[:])
                    rb = recipp.tile([P, NQ], F32, tag="rbb", bufs=3, name="rb")
                    nc.sync.dma_start(
                        out=rb[r0 : r0 + 64, :],
                        in_=bass.AP(rcd[:].tensor, rcd[:].offset, [[0, 64], [1, NQ]]),
                    )
                    tm = recipp.tile([P, NQ], F32, tag="tmn", bufs=2, name="tm")
                    nc.vector.tensor_mul(
                        tm[r0 : r0 + 64, :],
                        po[0:HD, :],
                        rb[r0 : r0 + 64, :],
                    )
                    nc.scalar.activation(
                        out=attnt[m][r0 : r0 + 64, q0:q1].bitcast(F32R),
                        in_=tm[r0 : r0 + 64, :],
                        func=mybir.ActivationFunctionType.Identity,
                        bias=bv_sb[r0 : r0 + 64, h // 2 : h // 2 + 1],
                        scale=1.0,
                    )

            # ---- Phase D: output projection (row-sharded Wo) ----
            for t in range(NK):
                for oc in range(2):
                    ps = pp.tile([P, 512], F32, tag="proj", bufs=2, name="ps_w")
                    for m in range(2):
                        nc.tensor.matmul(
                            ps[:],
                            attnt[m][:, t * P : (t + 1) * P].bitcast(F32R),
                            wo_sb[:, m, oc * 512 : (oc + 1) * 512],
                            start=(m == 0),
                            stop=(m == 1),
                        )
                    ob = constp.tile(  # alias dead cos/sin slots
                        [P, 512], F32, tag="ct" if (2 * t + oc) % 2 == 0 else "st",
                        name=f"ob{t}_{oc}",
                    )
                    nc.vector.tensor_copy(out=ob[:], in_=ps[:])
                    nc.sync.dma_start(
                        out=out[:, t, oc * 512 : (oc + 1) * 512], in_=ob[:]
                    )

    nc.compile()
    return nc


# ---------------- host-side prep ----------------

_PERM64 = np.concatenate([np.arange(0, HD, 2), np.arange(1, HD, 2)])


def _rope_tables():
    inv = 1.0 / (10000.0 ** (np.arange(0, HD, 2, dtype=np.float32) / HD))
    t = np.arange(S, dtype=np.float32)
    ang = np.outer(t, inv)  # [S, HD/2]
    return np.cos(ang).astype(np.float32), np.sin(ang).astype(np.float32)


def _tile_rows(a, p=P):
    """[R, N] -> [p, R//p, N] where row r = a[idx*p + ...]: r = a_row % p"""
    R = a.shape[0]
    return np.ascontiguousarray(
        a.reshape(R // p, p, *a.shape[1:]).transpose(1, 0, *range(2, a.ndim + 1))
    )


def shard_inputs(x, effective_len, Wq, bq, Wk, bk, Wv, bv, Wo, bo):
    x = np.asarray(x, np.float32)
    effective_len = np.asarray(effective_len, np.int32)
    Wq, Wk, Wv, Wo = (np.asarray(w, np.float32) for w in (Wq, Wk, Wv, Wo))
    bq, bk, bv = (np.asarray(b, np.float32) for b in (bq, bk, bv))

    cos, sin = _rope_tables()  # [S, 32]
    ct = np.ascontiguousarray(np.tile(cos.T, (4, 1)))  # [128, S]
    st = np.ascontiguousarray(np.tile(sin.T, (4, 1)))

    # causal multiplicative masks for the two diagonal key-tiles of a chunk
    kl = np.arange(P)[:, None]
    ql = np.arange(NQ)[None, :]
    cm = np.stack(
        [(ql >= kl).astype(np.float32), (ql >= kl + P).astype(np.float32)]
    )  # [2, 128, NQ]
    cmask = np.ascontiguousarray(cm.transpose(1, 0, 2))  # [128, 2, NQ]

    in_maps = []
    for b in range(B):
        xt = _tile_rows(np.ascontiguousarray(x[b].T))  # [128, 8, S]
        g_vec = (np.arange(S) < (S - int(effective_len[b]))).astype(np.float32)
        gate = np.ascontiguousarray(
            np.repeat(g_vec.reshape(NK, P).T[:, :, None], HL, axis=2)
        )  # [128, NK, HL]
        for g in range(G):
            rows = np.concatenate(
                [g * GF + h * HD + _PERM64 for h in range(HL)]
            )  # permuted head dims for Q/K
            vrows = np.arange(g * GF, (g + 1) * GF)
            in_maps.append(
                {
                    "xt": xt,
                    "wq": _tile_rows(np.ascontiguousarray(Wq[rows].T)),
                    "wk": _tile_rows(np.ascontiguousarray(Wk[rows].T)),
                    "wv": _tile_rows(np.ascontiguousarray(Wv[vrows].T)),
                    "wo": _tile_rows(np.ascontiguousarray(Wo[:, vrows].T)),
                    "bq": np.ascontiguousarray(bq[rows].reshape(2, P).T),
                    "bk": np.ascontiguousarray(bk[rows].reshape(2, P).T),
                    "bv": np.ascontiguousarray(bv[vrows].reshape(2, P).T),
                    "ct": ct,
                    "st": st,
                    "gate": gate,
                    "cmask": cmask,
                }
            )
    return in_maps


def gather_outputs(results, bo):
    bo = np.asarray(bo, np.float32)
    out = np.zeros((B, S, D), np.float32)
    for b in range(B):
        acc = np.zeros((S, D), np.float32)
        for g in range(G):
            o3 = results[b * G + g]["out"]  # [128, NK, D]
            acc += o3.transpose(1, 0, 2).reshape(S, D)
        out[b] = acc + bo
    return out


_NC_CACHE = None


def _get_nc():
    global _NC_CACHE
    if _NC_CACHE is None:
        _NC_CACHE = build_nc()
    return _NC_CACHE


def kernel(**inputs):
    from concourse.bass_utils import run_bass_kernel_spmd

    nc = _get_nc()
    in_maps = shard_inputs(**inputs)
    res = run_bass_kernel_spmd(nc, in_maps, core_ids=list(range(8)))
    return gather_outputs(res.results, inputs["bo"])
